# revision 9
# baseline (speedup 1.0000x reference)
"""Trainium2 Bass kernel for Causal ALIF layer 2D (spiking neural net scan).

Reference math (per element, scan over T):
    v      = v_prev * 0.8 + (x_t * gamma + beta)
    vth    = 0.5 + u                       (u = vth_dyn)
    s      = (v - vth) > 0 ? 1.0 : 0.0
    v_post = v - vth * s
    u'     = u * decay_eff + s * step_eff
    outputs per step: (s, v)   [v is pre-reset]

Sharding: data-parallel over batch B=16 across 8 cores (2 batches/core).
Per core the (h,w,c) space = 65536 elems = [128 partitions, 512 cols];
the 2 local batches sit side by side in columns -> [128, 1024] fp32 tiles.

Engine split (v2): the serial recurrence loop runs on DVE as 4 ops/step:
    cv:   v      = vp_prev*0.8 + x_t            (STT)
    cd:   d      = (v - 0.5) - u_prev           (STT)
    cs:   s      = d > 0                        (TSS is_gt, bf16 out)
    pred: v     <- d where s (copy_predicated)  == v_post, in place
(v_post == d exactly when s=1 since d = v - vth.)  The u-chain runs on the
otherwise-idle Pool engine (gpsimd):
    cud:  ud = u_prev * decay_eff               (TT)
    cu:   u  = s*step_eff + ud                  (STT)
and the ACT engine down-converts v to bf16 for the vlt store:
    cb:   vb = Copy(v) -> bf16
so the in-place pred only waits on the fast ACT copy, never on a store DMA
(DMA completion sem propagation is ~900ns).  Outputs are stored bf16
(spikes are exactly 0/1 in bf16; vlt bf16 quantization ~1e-3 rel) and
up-converted to fp32 on the host, halving store DMA traffic.

Raw bass (no Tile): this toolchain's walrus accepts at most ONE sync-wait
per compute instruction, so all waits are standalone wait_ge instructions.
Same-engine deps rely on in-order execution (no wait emitted); cross-engine
deps use per-engine completion counters (eng sems, +1 per op) and DMA sems
(+16 per transfer).  Per-slot DMA sems keep WAR correct even if DMA queues
complete out of order.
"""

import numpy as np

import concourse.bass as bass
import concourse.mybir as mybir
from concourse.bass_utils import run_bass_kernel_spmd

B, T, H, W, C = 16, 64, 32, 32, 64
DECAY_V = 0.8
VTH_BASE = 0.5
N_CORES = 8
B_LOC = B // N_CORES          # 2
P = 128                       # SBUF partitions
NB = H * W * C // P           # 512 per-batch columns
COLS = B_LOC * NB             # 1024 tile columns

XB = 4   # x-tile slots
SB = 3   # s-tile slots
VBF = 3  # vb (bf16 v) slots
VV = 3   # v-state slots

F32 = mybir.dt.float32
BF16 = mybir.dt.bfloat16
U8 = mybir.dt.uint8
OP = mybir.AluOpType
AF = mybir.ActivationFunctionType


def _dual(dram, g0, gl):
    """DRAM [P, COLS] param (same [P,NB] block per batch) -> AP covering
    per-batch cols [g0,g0+gl) of both batch blocks, ordered (p, b, n)."""
    return bass.AP(dram, g0, [[COLS, P], [NB, B_LOC], [1, gl]])


def _xap(dram, t, g0, gl):
    """x/spk/vlt DRAM [B_LOC, T, P, NB] slice [:, t, :, g0:g0+gl] as
    (p, b, n) to match SBUF [P, B_LOC*gl]."""
    off = t * P * NB + g0
    return bass.AP(
        dram,
        off,
        [[NB, P], [T * P * NB, B_LOC], [1, gl]],
    )


def _build_kernel(se_imm, use_gamma_beta, se_is_tensor, reps=1):
    """v2 builder: DVE recurrence loop + Pool u-chain + ACT bf16 copy.

    Emission model: every engine block runs the same deterministic planner
    (`plan(target)`) but only emits its own instructions.  Dependencies are
    (semaphore, value) tuples; same-engine deps are pruned (in-order exec),
    cross-engine waits are standalone wait_ge with a high-water mark per
    (engine, sem) to skip redundant waits.
    """
    from contextlib import ExitStack

    nc = bass.Bass(target_bir_lowering=False)

    x_d = nc.dram_tensor("x", [B_LOC, T, P, NB], F32, kind="ExternalInput")
    de_d = nc.dram_tensor("de", [P, COLS], F32, kind="ExternalInput")
    se_d = ga_d = be_d = None
    if se_is_tensor:
        se_d = nc.dram_tensor("se", [P, COLS], F32, kind="ExternalInput")
    if use_gamma_beta:
        ga_d = nc.dram_tensor("ga", [P, COLS], F32, kind="ExternalInput")
        be_d = nc.dram_tensor("be", [P, COLS], F32, kind="ExternalInput")
    spk_d = nc.dram_tensor("spk", [B_LOC, T, P, NB], U8, kind="ExternalOutput")
    vlt_d = nc.dram_tensor("vlt", [B_LOC, T, P, NB], BF16, kind="ExternalOutput")

    engine_names = ["vector", "gpsimd", "scalar"]
    NT = reps * T

    with ExitStack() as ctx:
        E = ctx.enter_context
        w = COLS
        de_t = E(nc.sbuf_tensor("de0", [P, w], F32))
        se_t = E(nc.sbuf_tensor("se0", [P, w], F32)) if se_is_tensor else None
        ga_t = E(nc.sbuf_tensor("ga0", [P, w], F32)) if use_gamma_beta else None
        be_t = E(nc.sbuf_tensor("be0", [P, w], F32)) if use_gamma_beta else None
        x_sb = [E(nc.sbuf_tensor(f"x{i}", [P, w], F32)) for i in range(XB)]
        s_sb = [E(nc.sbuf_tensor(f"s{i}", [P, w], U8)) for i in range(SB)]
        vb_sb = [E(nc.sbuf_tensor(f"vb{i}", [P, w], BF16)) for i in range(VBF)]
        v_sb = [E(nc.sbuf_tensor(f"v{i}", [P, w], F32)) for i in range(VV)]
        u_sb = [E(nc.sbuf_tensor(f"u{i}", [P, w], F32)) for i in range(2)]
        d_t = E(nc.sbuf_tensor("d0", [P, w], F32))
        ud_t = E(nc.sbuf_tensor("ud0", [P, w], F32))
        tse_t = E(nc.sbuf_tensor("tse0", [P, w], F32)) if se_is_tensor else None
        acc_t = E(nc.sbuf_tensor("acc0", [P, w], F32)) if use_gamma_beta else None

        prm_sem = E(nc.semaphore("prm"))
        x_sem = [E(nc.semaphore(f"xs{i}")) for i in range(XB)]
        sp_sem = [E(nc.semaphore(f"ss{i}")) for i in range(SB)]
        vb_sem = [E(nc.semaphore(f"vs{i}")) for i in range(VBF)]
        eng_sems = {nm: E(nc.semaphore(f"esem_{nm}")) for nm in engine_names}

        n_prm = 1 + (1 if se_is_tensor else 0) + (2 if use_gamma_beta else 0)

        # planner outputs for the sync program
        plan_done = [False]
        c_s = [None] * NT     # (sem, val) of spike producer (per gt)
        c_xfree = [None] * NT  # (sem, val) of last x-slot reader

        # ACT does exactly one Copy per step, so its ordinals are closed-form;
        # vector ordinals (c_v) are filled by the first plan() run (the
        # vector block runs before the scalar/sync blocks).
        vec_sem = eng_sems["vector"]
        sc_sem = eng_sems["scalar"]
        c_v = [None] * NT
        c_vb = [(sc_sem, gt + 1) for gt in range(NT)]

        def plan(target):
            ests = {nm: {"sem": eng_sems[nm], "n": 0, "hw": {}}
                    for nm in engine_names}

            def op(eng_name, emit_fn, waits):
                est = ests[eng_name]
                if eng_name == target:
                    eng = getattr(nc, eng_name)
                    for sem, val in waits:
                        if sem is est["sem"]:
                            continue  # same-engine dep: in-order exec
                        k = id(sem)
                        if est["hw"].get(k, 0) < val:
                            eng.wait_ge(sem, val)
                            est["hw"][k] = val
                    emit_fn(eng).then_inc(est["sem"], 1)
                else:
                    for sem, val in waits:
                        if sem is est["sem"]:
                            continue
                        k = id(sem)
                        if est["hw"].get(k, 0) < val:
                            est["hw"][k] = val
                est["n"] += 1
                return (est["sem"], est["n"])

            prm_w = (prm_sem, 16 * n_prm)
            cu_prev = None     # (sem,val) of cu(gt-1)
            cu_hist = [None] * NT
            for gt in range(NT):
                t = gt % T
                xi, si, bi, vi = gt % XB, gt % SB, gt % VBF, gt % VV
                pu, cui = (gt - 1) % 2, gt % 2
                v_t = v_sb[vi][:]
                v_prev = v_sb[(gt - 1) % VV][:]
                x_t = x_sb[xi][:]
                s_t = s_sb[si][:]
                d = d_t[:]

                x_wait = (x_sem[xi], 16 * (gt // XB + 1))

                # ---- VECTOR: recurrence loop -------------------------------
                if use_gamma_beta:
                    waits = [x_wait]
                    if gt == 0:
                        waits.append(prm_w)
                    a0 = op("vector", lambda e, a=acc_t[:], x=x_t, g=ga_t[:]:
                            e.tensor_tensor(a, x, g, op=OP.mult), waits)
                    a1 = op("vector", lambda e, a=acc_t[:], b=be_t[:]:
                            e.tensor_tensor(a, a, b, op=OP.add), [])
                    vin = acc_t[:]
                    cv_waits = []
                    c_xfree[gt] = a0
                else:
                    vin = x_t
                    cv_waits = [x_wait]
                # WAR: ACT's vb copy of step gt-VV read v slot vi
                if gt >= VV:
                    cv_waits.append(c_vb[gt - VV])
                if t == 0:
                    cv = op("vector", lambda e, v=v_t, a=vin:
                            e.tensor_copy(v, a), cv_waits)
                else:
                    cv = op("vector", lambda e, v=v_t, vp=v_prev, a=vin:
                            e.scalar_tensor_tensor(v, vp, DECAY_V, a,
                                                   OP.mult, OP.add), cv_waits)
                assert c_v[gt] is None or c_v[gt] == cv
                c_v[gt] = cv
                if not use_gamma_beta:
                    c_xfree[gt] = cv

                if t == 0:
                    cd = op("vector", lambda e, dd=d, v=v_t:
                            e.tensor_single_scalar(dd, v, VTH_BASE,
                                                   op=OP.subtract), [])
                else:
                    cd = op("vector", lambda e, dd=d, v=v_t, u=u_sb[pu][:]:
                            e.scalar_tensor_tensor(dd, v, VTH_BASE, u,
                                                   OP.subtract, OP.subtract),
                            [cu_prev])

                cs_waits = []
                if gt >= SB:
                    cs_waits.append((sp_sem[si], 16 * (gt // SB)))
                    cs_waits.append(cu_hist[gt - SB])
                cs = op("vector", lambda e, s=s_t, dd=d:
                        e.tensor_single_scalar(s, dd, 0.0, op=OP.is_gt),
                        cs_waits)
                c_s[gt] = cs

                # ---- u-chain: cud (TT) on Pool; cu back on DVE (walrus
                # rejects TensorScalarPtr on the Pool engine). ------------
                if t > 0:
                    gp_waits = []
                    if gt == 1:
                        gp_waits.append(prm_w)
                    # RAW u[pu] written by vector cu(gt-1); also covers the
                    # ud WAR (vector cu(gt-1) read ud before this overwrite).
                    gp_waits.append(cu_prev)
                    cud = op("gpsimd", lambda e, u=u_sb[pu][:], dd=de_t[:]:
                             e.tensor_tensor(ud_t[:], u, dd, op=OP.mult),
                             gp_waits)
                if se_is_tensor:
                    if t == 0:
                        cu_w = [prm_w] if gt == 0 else []
                        cu = op("vector", lambda e, u=u_sb[cui][:], s=s_t:
                                e.tensor_tensor(u, s, se_t[:], op=OP.mult),
                                cu_w)
                    else:
                        op("vector", lambda e, s=s_t:
                           e.tensor_tensor(tse_t[:], s, se_t[:], op=OP.mult),
                           [])
                        cu = op("vector", lambda e, u=u_sb[cui][:]:
                                e.tensor_tensor(u, tse_t[:], ud_t[:],
                                                op=OP.add), [cud])
                else:
                    if t == 0:
                        cu = op("vector", lambda e, u=u_sb[cui][:], s=s_t:
                                e.tensor_single_scalar(u, s, se_imm,
                                                       op=OP.mult), [])
                    else:
                        cu = op("vector", lambda e, u=u_sb[cui][:], s=s_t:
                                e.scalar_tensor_tensor(u, s, se_imm, ud_t[:],
                                                       OP.mult, OP.add),
                                [cud])
                cu_prev = cu
                cu_hist[gt] = cu

                # in-place reset: v <- d where spike.  Waits for this step's
                # ACT bf16 copy (the only other reader of the v tile).
                op("vector", lambda e, v=v_t, s=s_t, dd=d:
                   e.copy_predicated(v, s, dd), [c_vb[gt]])

            plan_done[0] = True

        # ACT program: one bf16 Copy of v per step (ordinals precomputed in
        # c_vb; asserted here).
        def act_plan():
            eng = nc.scalar
            hwm = {}

            def wait(dep):
                s, val = dep
                if s is sc_sem:
                    return
                k = id(s)
                if hwm.get(k, 0) < val:
                    eng.wait_ge(s, val)
                    hwm[k] = val

            for gt in range(NT):
                bi, vi = gt % VBF, gt % VV
                wait(c_v[gt])
                if gt >= VBF:
                    wait((vb_sem[bi], 16 * (gt // VBF)))
                eng.activation(
                    vb_sb[bi][:], v_sb[vi][:], AF.Copy
                ).then_inc(sc_sem, 1)

        with nc.Block() as block:
            @block.vector
            def _(eng):
                plan("vector")

            @block.gpsimd
            def _(eng):
                plan("gpsimd")

            @block.scalar
            def _(eng):
                act_plan()

            @block.sync
            def _(sync):
                assert plan_done[0]
                hw = {}

                def swait(dep):
                    sem, val = dep
                    k = id(sem)
                    if hw.get(k, 0) < val:
                        sync.wait_ge(sem, val)
                        hw[k] = val

                def ld(dst_tile, src_ap, sem):
                    sync.dma_start(
                        dst_tile[:].rearrange("p (b n) -> p b n", b=B_LOC),
                        src_ap,
                    ).then_inc(sem, 16)

                ld(de_t, _dual(de_d, 0, NB), prm_sem)
                if se_is_tensor:
                    ld(se_t, _dual(se_d, 0, NB), prm_sem)
                if use_gamma_beta:
                    ld(ga_t, _dual(ga_d, 0, NB), prm_sem)
                    ld(be_t, _dual(be_d, 0, NB), prm_sem)
                for gt in range(min(XB, NT)):
                    ld(x_sb[gt], _xap(x_d, gt % T, 0, NB), x_sem[gt])

                for gt in range(NT):
                    t = gt % T
                    si, bi = gt % SB, gt % VBF
                    # x load for step gt+XB (slot free after last read @ gt)
                    if gt + XB < NT:
                        swait(c_xfree[gt])
                        ld(x_sb[(gt + XB) % XB],
                           _xap(x_d, (gt + XB) % T, 0, NB),
                           x_sem[(gt + XB) % XB])
                    # vlt store (bf16 copy ready first)
                    swait(c_vb[gt])
                    sync.dma_start(
                        _xap(vlt_d, t, 0, NB),
                        vb_sb[bi][:].rearrange("p (b n) -> p b n", b=B_LOC),
                    ).then_inc(vb_sem[bi], 16)
                    # spk store
                    swait(c_s[gt])
                    sync.dma_start(
                        _xap(spk_d, t, 0, NB),
                        s_sb[si][:].rearrange("p (b n) -> p b n", b=B_LOC),
                    ).then_inc(sp_sem[si], 16)

    return nc


def _param_to_tile(p):
    """[H,W,C] -> [128, COLS]: [128, NB] block repeated for each batch."""
    m = np.ascontiguousarray(np.asarray(p, dtype=np.float32)).reshape(P, NB)
    return np.ascontiguousarray(np.tile(m, (1, B_LOC)))


_CACHE = {}
_BENCH_CACHE = {}


def _prepare(inputs, reps=1):
    x = np.asarray(inputs["x"], dtype=np.float32)
    hp_base_step = np.float32(inputs["hp_base_step"])
    hp_base_decay = np.float32(inputs["hp_base_decay"])
    step_w_raw = np.asarray(inputs["step_w_raw"], dtype=np.float32)
    decay_w_raw = np.asarray(inputs["decay_w_raw"], dtype=np.float32)
    gamma = np.asarray(inputs["gamma"], dtype=np.float32)
    beta = np.asarray(inputs["beta"], dtype=np.float32)

    # Effective params, computed to match the f32 jax ops in the reference.
    import jax
    import jax.numpy as jnp

    cpu = jax.devices("cpu")[0]
    with jax.default_device(cpu):
        step_w = np.asarray(jax.nn.softplus(jnp.asarray(step_w_raw)))
        decay_w = np.asarray(jax.nn.sigmoid(jnp.asarray(decay_w_raw)))
        se_full = np.asarray(jnp.float32(hp_base_step) * step_w)
        de_full = np.asarray(
            jnp.float32(hp_base_decay)
            + (jnp.float32(1.0) - jnp.float32(hp_base_decay)) * decay_w
        )

    use_gamma_beta = not (np.all(gamma == 1.0) and np.all(beta == 0.0))
    se_is_tensor = not np.all(se_full == se_full.flat[0])
    se_imm = float(se_full.flat[0])

    key = (
        se_imm if not se_is_tensor else None,
        use_gamma_beta,
        se_is_tensor,
        reps,
    )
    if key not in _CACHE:
        _CACHE[key] = _build_kernel(
            se_imm, use_gamma_beta, se_is_tensor, reps=reps
        )
    nc = _CACHE[key]

    de_tile = _param_to_tile(de_full)
    in_maps = []
    for i in range(N_CORES):
        m = {
            "x": np.ascontiguousarray(
                x[i * B_LOC : (i + 1) * B_LOC].reshape(B_LOC, T, P, NB)
            ),
            "de": de_tile,
        }
        if se_is_tensor:
            m["se"] = _param_to_tile(se_full)
        if use_gamma_beta:
            m["ga"] = _param_to_tile(gamma)
            m["be"] = _param_to_tile(beta)
        in_maps.append(m)
    return nc, in_maps


def _gather(res):
    spk = np.concatenate(
        [
            np.asarray(r["spk"]).astype(np.float32).reshape(B_LOC, T, H, W, C)
            for r in res.results
        ],
        axis=0,
    )
    vlt = np.concatenate(
        [
            np.asarray(r["vlt"]).astype(np.float32).reshape(B_LOC, T, H, W, C)
            for r in res.results
        ],
        axis=0,
    )
    return spk, vlt


def kernel(**inputs):
    nc, in_maps = _prepare(inputs)
    res = run_bass_kernel_spmd(nc, in_maps, core_ids=list(range(N_CORES)))
    return _gather(res)


def _make_sharded_fn(nc):
    """Replicate bass2jax.run_bass_via_pjrt's multi-core path, returning
    (fn, in_names, out_names, out_avals, mesh) with fn jitted over
    core-sharded global arrays; outputs donated from zero buffers."""
    import jax
    from jax.sharding import Mesh, PartitionSpec
    from jax.experimental.shard_map import shard_map

    from concourse import bass2jax, mybir as _mybir

    bass2jax.install_neuronx_cc_hook()
    partition_name = nc.partition_id_tensor.name if nc.partition_id_tensor else None
    in_names, out_names, out_avals, zero_outs = [], [], [], []
    for alloc in nc.m.functions[0].allocations:
        if not isinstance(alloc, _mybir.MemoryLocationSet):
            continue
        name = alloc.memorylocations[0].name
        if alloc.kind == "ExternalInput":
            if name != partition_name:
                in_names.append(name)
        elif alloc.kind == "ExternalOutput":
            shape = tuple(alloc.tensor_shape)
            dtype = _mybir.dt.np(alloc.dtype)
            out_names.append(name)
            out_avals.append(jax.core.ShapedArray(shape, dtype))
            zero_outs.append(np.zeros(shape, dtype))
    n_params = len(in_names)
    all_in_names = list(in_names) + list(out_names)
    if partition_name is not None:
        all_in_names.append(partition_name)
    donate = tuple(range(n_params, n_params + len(out_names)))

    def _body(*args):
        operands = list(args)
        if partition_name is not None:
            operands.append(bass2jax.partition_id_tensor())
        return tuple(
            bass2jax._bass_exec_p.bind(
                *operands,
                out_avals=tuple(out_avals),
                in_names=tuple(all_in_names),
                out_names=tuple(out_names),
                lowering_input_output_aliases=(),
                sim_require_finite=True,
                sim_require_nnan=True,
                nc=nc,
            )
        )

    devices = jax.devices()[:N_CORES]
    mesh = Mesh(np.asarray(devices), ("core",))
    in_specs = (PartitionSpec("core"),) * (n_params + len(out_names))
    out_specs = (PartitionSpec("core"),) * len(out_names)
    fn = jax.jit(
        shard_map(_body, mesh=mesh, in_specs=in_specs, out_specs=out_specs,
                  check_rep=False),
        donate_argnums=donate,
        keep_unused=True,
    )
    return fn, in_names, out_names, out_avals, mesh


def bench(inputs, iters=10, reps=1):
    """Wall-clock benchmark with device-resident inputs. Returns dict with
    per-iteration times (s); each timed region is exactly one sharded NEFF
    execution (fresh donated zero outputs are made outside the region)."""
    import time

    import jax
    import jax.numpy as jnp
    from jax.sharding import NamedSharding, PartitionSpec

    nc, in_maps = _prepare(inputs, reps=reps)
    ck = id(nc)
    if ck not in _BENCH_CACHE:
        fn, in_names, out_names, out_avals, mesh = _make_sharded_fn(nc)
        sh = NamedSharding(mesh, PartitionSpec("core"))
        concat_in = [
            np.concatenate(
                [np.asarray(in_maps[c][k]) for c in range(N_CORES)], axis=0
            )
            for k in in_names
        ]
        dev_in = [jax.device_put(a, sh) for a in concat_in]
        jax.block_until_ready(dev_in)
        zshapes = [(N_CORES * a.shape[0], *a.shape[1:]) for a in out_avals]
        zdtypes = [a.dtype for a in out_avals]
        zeros_fn = jax.jit(
            lambda: tuple(jnp.zeros(s, d) for s, d in zip(zshapes, zdtypes)),
            out_shardings=tuple(sh for _ in zshapes),
        )
        _BENCH_CACHE[ck] = (fn, dev_in, zeros_fn, out_names)
    fn, dev_in, zeros_fn, out_names = _BENCH_CACHE[ck]

    times = []
    out = None
    for i in range(iters):
        z = zeros_fn()
        jax.block_until_ready(z)
        t0 = time.perf_counter()
        out = fn(*dev_in, *z)
        jax.block_until_ready(out)
        times.append(time.perf_counter() - t0)
    res_out = {k: np.asarray(v) for k, v in zip(out_names, out)}
    return {"times": times, "out": res_out}


def measure(inputs, k=9, iters=14, rounds=6):
    """Estimate single-scan HW time via the slope between a reps=1 NEFF and
    a reps=k NEFF (k back-to-back identical scans inside one NEFF). The
    fixed dispatch/launch overhead cancels in the difference.  The two
    configs are INTERLEAVED over several rounds (launch overhead drifts by
    tens of ms under axon, so a single min-of-iters per config is not
    reliable); min-of-all suppresses host-side jitter."""
    t1s, tks = [], []
    r1 = rk = None
    for _ in range(rounds):
        r1 = bench(inputs, iters=iters, reps=1)
        rk = bench(inputs, iters=iters, reps=k)
        t1s.extend(r1["times"])
        tks.extend(rk["times"])
    t1 = min(t1s)
    tk = min(tks)
    ns = (tk - t1) / (k - 1) * 1e9
    r1["times"] = t1s
    rk["times"] = tks
    return ns, r1, rk
